# revision 7
# baseline (speedup 1.0000x reference)
"""Trainium2 Bass kernel for CoarseSkeletonHead.

Computes, for z [B, 256]:
    h = silu(z @ W1 + b1)                       # [B, 512]
    raw = (h @ W2 + b2).reshape(B, 23, 4)
    direction = normalize(raw[..., :3], eps=1e-6)
    length = softplus(raw[..., 3])
    offsets = [0, direction * length]           # [B, 24, 3]
    joints = einsum('jk,bkc->bjc', ANC, offsets)
returns (joints, offsets, length).

Strategy: pure data parallelism over 8 cores (B/8 rows each). Feature-major
(transposed) dataflow on-chip: the host pre-transposes z so every matmul
consumes naturally-laid-out operands and no on-chip transposes are needed.
The per-joint head math uses a quadrant partition layout (dx/dy/dz/len at
partition offsets 0/32/64/96) so all compute-engine SBUF operands start at
legal partition offsets; the tensor engine does all cross-partition routing
via small static 0/1 matrices (component-sum, scale replication, forward
kinematics). ScalarE uses only two activation table sets: Silu, and
exp/ln (softplus = ln(1+exp(x)), rsqrt = exp(-0.5*ln(max(nsq, 1e-12)))).
"""

from contextlib import ExitStack

import numpy as np

import concourse.bass as bass
import concourse.tile as tile
from concourse import bacc, mybir
from concourse.bass_utils import run_bass_kernel_spmd

F32 = mybir.dt.float32
AF = mybir.ActivationFunctionType

N_CORES = 8
B_FULL = 131072
D_IN = 256
D_HID = 512
J = 24
NJ = J - 1  # 23 non-root joints

PARENT = [-1, 0, 0, 0, 1, 2, 3, 4, 5, 6, 7, 8, 9, 9, 9, 12, 13, 14, 16, 17,
          18, 19, 20, 21]

TILE_B = 128          # batch columns per matmul tile
SB_B = 512            # batch columns per superblock (4 tiles, one PSUM bank)
PACK_SBS = 4          # superblocks packed (as partition quadrants) per pack
ZGRP_SBS = 2          # superblocks per z-load DMA (1 MiB)

_EPS_NSQ = 1e-12      # max(norm, 1e-6) == sqrt(max(nsq, 1e-12))


def _ancestor_matrix():
    A = np.zeros((J, J), dtype=np.float32)
    for j in range(1, J):
        k = j
        while k > 0:
            A[j, k] = 1.0
            k = PARENT[k]
    return A


def _static_mats():
    """Build the static routing matrices (quadrant layout q*32+j)."""
    anc = _ancestor_matrix()
    # S [87, 32]: nsq[j] = sum_q<3 sq[32q+j]; cols 23:32 write zeros into
    # the pack-bank gap rows so later full-width ops read defined data.
    S = np.zeros((87, 32), dtype=np.float32)
    for q in range(3):
        for j in range(NJ):
            S[32 * q + j, j] = 1.0
    # R4 [119, 96]: replicate scale (at quadrant s) to 3 dir quadrants
    # (cols 87:96 stay zero so offT rows 87:96 come out as defined zeros).
    R4 = np.zeros((119, 96), dtype=np.float32)
    for s in range(4):
        for j in range(NJ):
            base = 32 * s + j
            if base >= 119:
                continue
            for q in range(3):
                R4[base, 32 * q + j] = 1.0
    # G4 [87, 72]: joints[3j+c] = sum_k ANC[j,k] * off[32c + (k-1)]
    G4 = np.zeros((87, 72), dtype=np.float32)
    for q in range(3):
        for j in range(J):
            for k in range(1, J):
                G4[32 * q + (k - 1), 3 * j + q] = anc[j, k]
    return S, R4, G4


def _permute_w2(W2, b2):
    """[512, 92] -> quadrant c-major [512, 119]; b2 -> [1, 119]."""
    W2q = np.zeros((D_HID, 119), dtype=np.float32)
    b2q = np.zeros((1, 119), dtype=np.float32)
    for q in range(4):
        for j in range(NJ):
            W2q[:, 32 * q + j] = W2[:, 4 * j + q]
            b2q[0, 32 * q + j] = b2[4 * j + q]
    return W2q, b2q


def build_nc(bc, b1_nonzero=False, b2_nonzero=False):
    """Build + compile the per-core Bass program for bc batch rows."""
    assert bc % (SB_B * PACK_SBS) == 0
    n_tiles = bc // TILE_B
    n_sb = n_tiles // 4
    n_pack = n_sb // PACK_SBS

    nc = bacc.Bacc("TRN2", target_bir_lowering=False, debug=False,
                   num_devices=N_CORES)

    zT = nc.dram_tensor("zT", [D_IN, bc], F32, kind="ExternalInput").ap()
    w1d = nc.dram_tensor("W1", [D_IN, D_HID], F32, kind="ExternalInput").ap()
    w2d = nc.dram_tensor("W2q", [D_HID, 119], F32, kind="ExternalInput").ap()
    sd = nc.dram_tensor("Smat", [87, 32], F32, kind="ExternalInput").ap()
    r4d = nc.dram_tensor("R4", [119, 96], F32, kind="ExternalInput").ap()
    g4d = nc.dram_tensor("G4", [87, 72], F32, kind="ExternalInput").ap()
    if b1_nonzero:
        b1d = nc.dram_tensor("b1c", [128, 4], F32, kind="ExternalInput").ap()
    if b2_nonzero:
        b2d = nc.dram_tensor("b2q", [1, 119], F32, kind="ExternalInput").ap()

    jointsq = nc.dram_tensor("jointsq", [72, bc], F32,
                             kind="ExternalOutput").ap()
    offq = nc.dram_tensor("offq", [96, bc], F32, kind="ExternalOutput").ap()
    lenq = nc.dram_tensor("lenq", [128, n_pack * SB_B], F32,
                      kind="ExternalOutput").ap()

    with tile.TileContext(nc) as tc, ExitStack() as ctx:
        singles = ctx.enter_context(tc.tile_pool(name="singles", bufs=1))
        zpool = ctx.enter_context(tc.tile_pool(name="zpool", bufs=2))
        hpool = ctx.enter_context(tc.tile_pool(name="hpool", bufs=2))
        dirp = ctx.enter_context(tc.tile_pool(name="dirp", bufs=6))
        sqp = ctx.enter_context(tc.tile_pool(name="sqp", bufs=2))
        offp = ctx.enter_context(tc.tile_pool(name="offp", bufs=3))
        s3p = ctx.enter_context(tc.tile_pool(name="s3p", bufs=2))
        jsbp = ctx.enter_context(tc.tile_pool(name="jsbp", bufs=2))
        packp = ctx.enter_context(tc.tile_pool(name="packp", bufs=2))
        lrp = ctx.enter_context(tc.tile_pool(name="lrp", bufs=2))
        ps_h = ctx.enter_context(tc.tile_pool(name="ps_h", bufs=2,
                                              space="PSUM"))
        ps_raw = ctx.enter_context(tc.tile_pool(name="ps_raw", bufs=2,
                                                space="PSUM"))
        ps_nsq = ctx.enter_context(tc.tile_pool(name="ps_nsq", bufs=2,
                                                space="PSUM"))
        ps_s3 = ctx.enter_context(tc.tile_pool(name="ps_s3", bufs=1,
                                               space="PSUM"))
        ps_j = ctx.enter_context(tc.tile_pool(name="ps_j", bufs=1,
                                              space="PSUM"))

        # ---- statics ----
        w1 = singles.tile([128, 2, D_HID], F32)
        nc.sync.dma_start(out=w1[:],
                          in_=w1d.rearrange("(c p) h -> p c h", p=128))
        w2 = singles.tile([128, 4, 119], F32)
        nc.sync.dma_start(out=w2[:],
                          in_=w2d.rearrange("(c p) m -> p c m", p=128))
        smat = singles.tile([87, 32], F32)
        nc.sync.dma_start(out=smat[:], in_=sd)
        r4 = singles.tile([119, 96], F32)
        nc.sync.dma_start(out=r4[:], in_=r4d)
        g4 = singles.tile([87, 72], F32)
        nc.sync.dma_start(out=g4[:], in_=g4d)
        if b1_nonzero:
            b1sb = singles.tile([128, 4], F32)
            nc.sync.dma_start(out=b1sb[:], in_=b1d)
        if b2_nonzero:
            b2sb = singles.tile([1, 119], F32)
            nc.sync.dma_start(out=b2sb[:], in_=b2d)
            ones = singles.tile([1, SB_B], F32)
            nc.vector.memset(ones[:], 1.0)

        zt_r = zT.rearrange("(c p) b -> p c b", p=128)

        for p in range(n_pack):
            nsq_ps = ps_nsq.tile([128, SB_B], F32)
            lenraw = lrp.tile([128, SB_B], F32)
            nc.vector.memset(lenraw[:], 0.0)
            dir_tiles = []
            for s in range(PACK_SBS):
                g = p * PACK_SBS + s
                if g % ZGRP_SBS == 0:
                    zsb = zpool.tile([128, 2, ZGRP_SBS * SB_B], F32)
                    c0 = g * SB_B
                    nc.sync.dma_start(
                        out=zsb[:],
                        in_=zt_r[:, :, c0:c0 + ZGRP_SBS * SB_B])
                ht = hpool.tile([128, 4, SB_B], F32)
                for tl in range(4):
                    toff = ((g % ZGRP_SBS) * 4 + tl) * TILE_B
                    hps = ps_h.tile([128, 512], F32)
                    for c in range(4):
                        for d in range(2):
                            nc.tensor.matmul(
                                hps[:, c * 128:(c + 1) * 128],
                                w1[:, d, c * 128:(c + 1) * 128],
                                zsb[:, d, toff:toff + TILE_B],
                                start=(d == 0), stop=(d == 1))
                    if b1_nonzero:
                        for c in range(4):
                            nc.scalar.activation(
                                ht[:, c, tl * TILE_B:(tl + 1) * TILE_B],
                                hps[:, c * 128:(c + 1) * 128],
                                AF.Silu, bias=b1sb[:, c:c + 1])
                    else:
                        nc.scalar.activation(
                            ht[:, :, tl * TILE_B:(tl + 1) * TILE_B],
                            hps[:].rearrange("p (c b) -> p c b", c=4),
                            AF.Silu)
                raw = ps_raw.tile([119, SB_B], F32)
                for c in range(4):
                    nc.tensor.matmul(
                        raw[:], w2[:, c, :], ht[:, c, :],
                        start=(c == 0),
                        stop=(c == 3 and not b2_nonzero))
                if b2_nonzero:
                    nc.tensor.matmul(raw[:], b2sb[:], ones[:],
                                     start=False, stop=True)
                dirT = dirp.tile([96, SB_B], F32)
                nc.vector.tensor_copy(dirT[:], raw[0:96, :])
                sqT = sqp.tile([87, SB_B], F32)
                nc.vector.tensor_mul(sqT[:], dirT[0:87, :], dirT[0:87, :])
                nc.tensor.matmul(nsq_ps[32 * s:32 * s + 32, :],
                                 smat[:], sqT[:], start=True, stop=True,
                                 tile_position=(0, 32 * s))
                nc.vector.tensor_copy(lenraw[32 * s:32 * s + NJ, :],
                                      raw[96:119, :])
                dir_tiles.append(dirT)

            # ---- pack-wide scalar chain (quadrant-packed, width SB_B) ----
            mT = packp.tile([128, SB_B], F32, tag="mT")
            nc.vector.tensor_scalar_max(mT[:], nsq_ps[:], _EPS_NSQ)
            lnm = packp.tile([128, SB_B], F32, tag="lnm")
            nc.scalar.activation(lnm[:], mT[:], AF.Ln)
            fac = packp.tile([128, SB_B], F32, tag="fac")
            nc.scalar.activation(fac[:], lnm[:], AF.Exp, scale=-0.5)
            eT = packp.tile([128, SB_B], F32, tag="eT")
            nc.scalar.activation(eT[:], lenraw[:], AF.Exp)
            lenT = packp.tile([128, SB_B], F32, tag="lenT")
            nc.scalar.activation(lenT[:], eT[:], AF.Ln, bias=1.0)
            scT = packp.tile([128, SB_B], F32, tag="scT")
            nc.vector.tensor_mul(scT[:], lenT[:], fac[:])

            nc.sync.dma_start(out=lenq[:, p * SB_B:(p + 1) * SB_B],
                              in_=lenT[:])

            # ---- deferred per-superblock tail ----
            for s in range(PACK_SBS):
                g = p * PACK_SBS + s
                gcol = g * SB_B
                s3ps = ps_s3.tile([96, SB_B], F32)
                nc.tensor.matmul(s3ps[:],
                                 r4[32 * s:32 * s + NJ, :],
                                 scT[32 * s:32 * s + NJ, :],
                                 start=True, stop=True,
                                 tile_position=(32 * s, 0))
                s3sb = s3p.tile([96, SB_B], F32)
                nc.vector.tensor_copy(s3sb[:], s3ps[:])
                offT = offp.tile([96, SB_B], F32)
                nc.vector.tensor_mul(offT[:], dir_tiles[s][:], s3sb[:])
                jps = ps_j.tile([72, SB_B], F32)
                nc.tensor.matmul(jps[:], g4[:], offT[0:87, :],
                                 start=True, stop=True)
                jsb = jsbp.tile([72, SB_B], F32)
                nc.vector.tensor_copy(jsb[:], jps[:])
                nc.sync.dma_start(out=jointsq[:, gcol:gcol + SB_B],
                                  in_=jsb[:])
                nc.sync.dma_start(out=offq[:, gcol:gcol + SB_B],
                                  in_=offT[:])

    nc.compile()
    return nc


def _host_prep(z, W1, b1, W2, b2, n_cores=N_CORES):
    b = z.shape[0]
    bc = b // n_cores
    S, R4, G4 = _static_mats()
    W2q, b2q = _permute_w2(np.asarray(W2, np.float32),
                           np.asarray(b2, np.float32))
    b1c = np.ascontiguousarray(
        np.asarray(b1, np.float32).reshape(4, 128).T)
    base = {
        "W1": np.ascontiguousarray(np.asarray(W1, np.float32)),
        "W2q": W2q, "Smat": S, "R4": R4, "G4": G4,
    }
    b1_nonzero = bool(np.any(b1))
    b2_nonzero = bool(np.any(b2))
    in_maps = []
    z = np.asarray(z, np.float32)
    for m in range(n_cores):
        im = dict(base)
        im["zT"] = np.ascontiguousarray(z[m * bc:(m + 1) * bc].T)
        if b1_nonzero:
            im["b1c"] = b1c
        if b2_nonzero:
            im["b2q"] = b2q
        in_maps.append(im)
    return in_maps, bc, b1_nonzero, b2_nonzero


def _assemble(results, b, bc):
    joints = np.empty((b, J, 3), np.float32)
    offsets = np.zeros((b, J, 3), np.float32)
    length = np.empty((b, NJ), np.float32)
    n_pack = bc // (PACK_SBS * SB_B)
    for m, r in enumerate(results):
        sl = slice(m * bc, (m + 1) * bc)
        joints[sl] = r["jointsq"].T.reshape(bc, J, 3)
        offsets[sl, 1:, :] = (r["offq"].reshape(3, 32, bc)[:, :NJ]
                              .transpose(2, 1, 0))
        # lenq [128, n_pack*512]: row 32s+j, col 512p+c <-> batch (4p+s)*512+c
        L = r["lenq"].reshape(PACK_SBS, 32, n_pack, SB_B)
        length[sl] = L.transpose(2, 0, 3, 1)[..., :NJ].reshape(bc, NJ)
    return joints, offsets, length


_NC_CACHE = {}


def _get_nc(bc, b1nz, b2nz):
    key = (bc, b1nz, b2nz)
    if key not in _NC_CACHE:
        _NC_CACHE[key] = build_nc(bc, b1nz, b2nz)
    return _NC_CACHE[key]


def run_sharded(z, W1, b1, W2, b2, trace=False):
    in_maps, bc, b1nz, b2nz = _host_prep(z, W1, b1, W2, b2)
    nc = _get_nc(bc, b1nz, b2nz)
    res = run_bass_kernel_spmd(nc, in_maps, core_ids=list(range(N_CORES)),
                               trace=trace)
    outs = _assemble(res.results, z.shape[0], bc)
    return outs, res


def kernel(z, W1, b1, W2, b2):
    outs, _ = run_sharded(z, W1, b1, W2, b2)
    return outs
